# revision 22
# baseline (speedup 1.0000x reference)
"""Cross-attention (B=2, T=2048, E=1024, H=16, Dh=64) on 8 trn2 NeuronCores.

Sharding: core c = b*4 + g  ->  batch b in {0,1}, head-group g in {0..3}
(4 heads per core).  Each core computes its heads' Q/K/V projections,
attention, and a partial out-projection; the host sums the 4 head-group
partials per batch (the tensor-parallel all-reduce, done at unshard
time) and adds the bias correction  bo + Wo @ bv.

Layout strategy: activations are fed to the device pre-transposed
([E, T] instead of [T, E]) so every matmul operand has its contraction
dim on partitions with no on-chip transposes.  Weights are fed as
Wq/Wk/Wv row-slices transposed ([E, 256]) and the Wo column-slice
transposed ([256, E]).

Attention math per head (no max-subtraction needed: |scores| <~ 10):
  S^T[k,q]  = K @ Q^T                (lhsT = K^T tile, rhs = Q^T tile;
              two heads share the PE array via row tiling at
              partition bases 0 and 64)
  Ex        = exp(S^T / 8)           (ScalarE, scale folded into exp,
              1024-wide over two k-tiles to amortize op overhead)
  PV[d,q]   = sum_k V_aug[k,d] * Ex[k,q]   (V_aug has a ones column ->
              row 64 accumulates the softmax denominator)
  A^T[d,q]  = PV[0:64,q] * (1 / PV[64,q])  (denominator broadcast to 64
              partitions with a K=1 matmul against a ones row)
  out      += A^T.T @ WoT_slice      (partial; host sums over groups)

Optimization history (NTFF-profiled NEFF exec time, core 0):
  f32r baseline                          ~426 us  (rel err 3.8e-04)
  + on-chip ones-column fill (the DMA    ~407 us
    broadcast was 8192 4B packets that
    blocked input streaming for ~76us)
  + all-bf16 matmul tiles (f32r tripped  ~314 us  (rel err 6.1e-03)
    the PE power throttle: util clamped
    to ~50-65% for 60% of the kernel)
  + approx reciprocal + spread out-proj  ~301 us  (rel err 6.3e-03)
    over following steps (power spikes)
    + bf16 output DMA
  + partition-major pre-shuffled inputs  ~292 us
    (1 descriptor/partition DMAs)
  + ACT-engine out-proj copies, OFF=5    ~288 us
Remaining wall: PE active ~250us of the ~290us span, with the DVFS
power throttle holding matmuls at ~1.35GHz (379ns/512-row matmul vs
213ns at the 2.4GHz peak; all 8 cores share the package power budget).
CA_PREC=f32r restores the full-precision path (~1.4x slower).
"""

import numpy as np

import concourse.bacc as bacc
import concourse.mybir as mybir
import concourse.tile as tile
from concourse.bass_utils import run_bass_kernel_spmd

# ---- custom DVE op: Schraudolph exp to bf16 bit pattern, one DVE pass ----
# j = round(x*(128/ln2)*SCALE + (16256 - C)) computed in fp32 via the
# magic-addend trick (adding 1.5*2^23 rounds to integer ULP); stored to an
# int16 view of the bf16 e-tile, whose bit pattern IS bf16(exp(x*SCALE)).
# Element rel err ~4.6% RMS (mantissa linear-interp sawtooth); diluted by
# sqrt(f) when only a fraction f of k-tiles use it, and further by softmax
# averaging.  Offloads exp from the saturated ACT engine to the DVE.
import concourse.dve_ops as _dve_ops
from concourse.dve_spec import C0 as _C0, C1 as _C1, C2 as _C2, Spec as _Spec, Src0 as _Src0
from concourse.dve_ops import DveOp as _DveOp

_LN2 = float(np.log(2.0))
_MAGIC = float(1.5 * 2**23)
_SCHC = 7.0  # Schraudolph bias: centers the sawtooth so DVE-exp tiles are
# unbiased relative to exact ACT-exp tiles (the bias, not the noise, is
# what hurts when the two are mixed within one softmax row)


def _exp2_bits_ref(in0, in1, s0, s1, imm2):
    t = (in0.astype(np.float32) * np.float32(s0)).astype(np.float32)
    t = (t + np.float32(s1)).astype(np.float32)
    return (t - np.float32(imm2)).astype(np.float32)


for _op in _dve_ops.OPS:
    if _op.name == "EXP2_BITS_ANT":
        EXP2_BITS_ANT = _op
        break
else:
    EXP2_BITS_ANT = _DveOp(
        "EXP2_BITS_ANT",
        _Spec(body=(_Src0 * _C0 + _C1) - _C2, reference=_exp2_bits_ref),
        subdim=False,
        uops_sha={"v3": "d08b4dae0546ff96", "v4": "78111510d711fb37"},
    )
    _dve_ops.OPS.append(EXP2_BITS_ANT)
    _dve_ops.CUSTOM_DVE_SPECS[EXP2_BITS_ANT.name] = EXP2_BITS_ANT.spec
    _dve_ops._SUB_OPCODE_FOR_NAME[EXP2_BITS_ANT.name] = (
        _dve_ops._CUSTOM_DVE_ROW_BASE + len(_dve_ops.OPS) - 1
    )

E = 1024          # embed dim
T = 2048          # seq len (q and k)
DH = 64           # head dim
DLOC = 256        # per-core projected dim (4 heads * 64)
ET = E // 128     # 8 contraction tiles over embed
KT = T // 128     # 16 k-tiles
KTP = KT // 2     # 8 k-tile pairs (exp processes 1024-wide)
TB = T // 128     # 16 t-blocks
QB = T // 512     # 4 q-blocks of 512
PAIRS = 2         # head pairs per core (2 heads share the 128 partitions)
SCALE = float(1.0 / np.sqrt(DH))

F32 = mybir.dt.float32
# Matmul compute dtype.  f32r streams 1 row/cycle like bf16, but the
# fp32 MAC power draw trips the DVFS throttle (measured: util clamped
# to ~50-65% for 60% of the kernel, ~2x on every matmul).  bf16 runs
# unthrottled and halves input DMA; measured rel err stays ~4e-3
# (gate 2e-2).  CA_PREC=f32r restores the full-precision path.
import os as _os

PREC = _os.environ.get("CA_PREC", "bf16")
MM_DT = mybir.dt.bfloat16 if PREC == "bf16" else mybir.dt.float32r


CDT = MM_DT  # dtype for every tile that feeds a matmul
USE_BF16_X = PREC == "bf16"
XDT = MM_DT
# Output dtype: bf16 halves the 8MB/core output DMA and the end-of-
# kernel drain; the host upcasts before summing the 4 partials.
ODT = mybir.dt.bfloat16 if PREC == "bf16" else F32


def build_nc(nrep=1, loop_n=0, timing=False, parts="xavbc", exp_op="exp", epool_bufs=10, off=5):
    # timing=True: big inputs become Internal (device-resident, unbound) so
    # the measurement harness ships ~nothing per dispatch; loop_n>0 wraps
    # the body in an on-device For_i loop to amplify exec time over the
    # dispatch noise.
    nc = bacc.Bacc("TRN2", target_bir_lowering=False, debug=False)

    # All big inputs are host-pre-shuffled to partition-major layouts so
    # every DMA lands as one large contiguous descriptor per partition
    # (8x fewer packets; the 2KB+ descriptors hit peak HBM throughput).
    big = "Internal" if timing else "ExternalInput"
    xq_d = nc.dram_tensor("xq", [128, 4 * ET * 512], XDT, kind=big)
    xk_d = nc.dram_tensor("xk", [128, 4 * ET * 512], XDT, kind=big)
    xv_d = nc.dram_tensor("xv", [128, 4 * ET * 512], XDT, kind=big)
    wq_d = nc.dram_tensor("wqt", [128, ET * DLOC], XDT, kind=big)
    wk_d = nc.dram_tensor("wkt", [128, ET * DLOC], XDT, kind=big)
    wv_d = nc.dram_tensor("wvt", [128, ET * DLOC], XDT, kind=big)
    wo_d = nc.dram_tensor("wot", [128, 2 * E], CDT, kind=big)
    bq_d = nc.dram_tensor("bq", [128, 2], F32, kind="ExternalInput")
    bk_d = nc.dram_tensor("bk", [128, 2], F32, kind="ExternalInput")
    out_d = nc.dram_tensor("out", [T, E], ODT, kind="ExternalOutput")

    import concourse.bass as bass
    from contextlib import ExitStack

    with tile.TileContext(nc) as tc:
        with ExitStack() as ctx:
            persist = ctx.enter_context(tc.tile_pool(name="persist", bufs=1))
            wpool = ctx.enter_context(tc.tile_pool(name="wpool", bufs=1))
            xpool = ctx.enter_context(tc.tile_pool(name="xpool", bufs=6))
            epool = ctx.enter_context(tc.tile_pool(name="epool", bufs=epool_bufs))
            spool = ctx.enter_context(tc.tile_pool(name="spool", bufs=2))
            opool = ctx.enter_context(tc.tile_pool(name="opool", bufs=2))
            # PSUM: 8 banks.  tag "s" = 2 slots of [128,1024] (2 banks
            # each: S-pair outputs; also projections and bc, briefly); tag
            # "pv" = 4 single-bank slots (PV accumulators for two in-flight
            # blocks, and out-proj accumulators).
            psum = ctx.enter_context(
                tc.tile_pool(name="psum", bufs=2, space="PSUM")
            )

            # Of the 8 ktp steps per block, how many of hh1's exps go to the
            # DVE Schraudolph op (ACT does the rest).  4 -> 25% of all exps.
            n8 = int(_os.environ.get("CA_DVE_EXP", "4"))
            if PREC != "bf16":
                n8 = 0
            dve_ktps = (
                set() if n8 <= 0 else {int(round(i * 8 / n8)) % 8 for i in range(n8)}
            )
            WARM = int(_os.environ.get("CA_WARM", "24"))

            def _body():
                # ---- PE warm-up ----
                # The HAM clock gate starts at K=4/8 (1.2GHz) and needs
                # ~3.4us of sustained PE activity to open up; the real work
                # is DMA-gated for the first ~10us.  Dummy matmuls on
                # zeroed tiles warm the clock so phase A runs at 2.4GHz.
                if WARM:
                    wu_w = persist.tile([128, 128], F32, name="wu_w", tag="wu_w")
                    wu_x = persist.tile([128, 512], F32, name="wu_x", tag="wu_x")
                    nc.vector.memset(wu_w[:], 0.0)
                    nc.vector.memset(wu_x[:], 0.0)
                    for i in range(WARM):
                        wu_ps = psum.tile(
                            [128, 512], F32, name=f"wu{i}", tag="pv", bufs=4
                        )
                        nc.tensor.matmul(
                            wu_ps[:], wu_w[:], wu_x[:], start=True, stop=True
                        )

                # ---- persistent tiles ----
                qT = [
                    persist.tile([128, T], CDT, name=f"qT{p}", tag=f"qT{p}")
                    for p in range(PAIRS)
                ]
                kTt = [
                    persist.tile([128, T], CDT, name=f"kT{p}", tag=f"kT{p}")
                    for p in range(PAIRS)
                ]
                aT = [
                    persist.tile([128, T], CDT, name=f"aT{p}", tag=f"aT{p}")
                    for p in range(PAIRS)
                ]
                # V is augmented with 64 ones-columns (cols DH..2*DH-1): the
                # PV matmul then emits the softmax denominator REPLICATED on
                # PSUM partitions 64..127 for free (matmul cost is N cycles
                # regardless of M), so normalization needs no partition
                # broadcast at all -- DVE reciprocal on [64,512] is barely
                # slower than [1,512] (it was single-lane before).
                v_t = persist.tile(
                    [128, KT, 4, 2 * DH], CDT, name="v_t", tag="v_t"
                )
                bq_t = persist.tile([128, 2], F32, name="bq_t", tag="bq_t")
                bk_t = persist.tile([128, 2], F32, name="bk_t", tag="bk_t")

                # Fill the ones-columns on-chip (a DMA broadcast here is
                # scattered 4B packets that monopolize the DMA rings; memset
                # can't emit f32r, so stage F32 and DVE-copy for that path).
                if PREC == "bf16":
                    nc.vector.memset(v_t[:, :, :, DH : 2 * DH], 1.0)
                else:
                    ones_col = persist.tile(
                        [128, KT * 4 * DH], F32, name="ones_col", tag="ones_col"
                    )
                    nc.vector.memset(ones_col[:], 1.0)
                    nc.vector.tensor_copy(
                        out=v_t[:, :, :, DH : 2 * DH],
                        in_=ones_col.rearrange(
                            "p (k h d) -> p k h d", k=KT, h=4
                        ),
                    )

                wq_t = wpool.tile([128, ET, DLOC], XDT, name="wq_t", tag="wq")
                wk_t = wpool.tile([128, ET, DLOC], XDT, name="wk_t", tag="wk")
                wv_t = wpool.tile([128, ET, DLOC], XDT, name="wv_t", tag="wv")
                wo_t = wpool.tile([128, 2, E], CDT, name="wo_t", tag="wo")

                # ---- phase A: projections (quarter x-tiles of 512 cols) ----
                # q/k/v interleaved per quart so all three input streams
                # prefetch ahead (the baseline's serial q,k,v order left a
                # ~3.9us PE gap waiting for xv -- one HAM MID window is
                # enough to re-throttle the PE clock to 1.2GHz).
                nc.sync.dma_start(
                    out=wq_t[:],
                    in_=wq_d.ap().rearrange("p (n m) -> p n m", n=ET),
                )
                nc.sync.dma_start(out=bq_t[:], in_=bq_d.ap())
                nc.sync.dma_start(
                    out=wk_t[:],
                    in_=wk_d.ap().rearrange("p (n m) -> p n m", n=ET),
                )
                nc.sync.dma_start(out=bk_t[:], in_=bk_d.ap())
                nc.sync.dma_start(
                    out=wv_t[:],
                    in_=wv_d.ap().rearrange("p (n m) -> p n m", n=ET),
                )
                nc.sync.dma_start(
                    out=wo_t[:],
                    in_=wo_d.ap().rearrange("p (n m) -> p n m", n=2),
                )
                QW = ET * 512
                for quart in range(4 if "x" in parts else 0):
                    csl = slice(quart * 512, (quart + 1) * 512)
                    xhs = {}
                    for nm, x_d in (("q", xq_d), ("k", xk_d), ("v", xv_d)):
                        xh = xpool.tile(
                            [128, ET, 512], XDT, name=f"x_{nm}{quart}", tag="x"
                        )
                        nc.sync.dma_start(
                            out=xh[:],
                            in_=x_d.ap()[
                                :, quart * QW : (quart + 1) * QW
                            ].rearrange("p (n c) -> p n c", n=ET),
                        )
                        xhs[nm] = xh
                    if "a" not in parts:
                        continue
                    for nm, w_t, b_t, dst in (
                        ("q", wq_t, bq_t, qT),
                        ("k", wk_t, bk_t, kTt),
                    ):
                        xh = xhs[nm]
                        for m in range(PAIRS):
                            ps = psum.tile(
                                [128, 512],
                                F32,
                                name=f"ps_{nm}{quart}{m}",
                                tag="s",
                            )
                            for et in range(ET):
                                nc.tensor.matmul(
                                    ps[:],
                                    w_t[:, et, m * 128 : (m + 1) * 128],
                                    xh[:, et, :],
                                    start=(et == 0),
                                    stop=(et == ET - 1),
                                )
                            nc.vector.tensor_scalar_add(
                                out=dst[m][:, csl],
                                in0=ps[:],
                                scalar1=b_t[:, m : m + 1],
                            )
                    xh = xhs["v"]
                    for tbl in range(4 if "v" in parts else 0):
                        tb = quart * 4 + tbl
                        ps = psum.tile([128, DLOC], F32, name=f"ps_v{tb}", tag="s")
                        for et in range(ET):
                            nc.tensor.matmul(
                                ps[:],
                                xh[:, et, tbl * 128 : (tbl + 1) * 128],
                                wv_t[:, et, :],
                                start=(et == 0),
                                stop=(et == ET - 1),
                            )
                        nc.vector.tensor_copy(
                            out=v_t[:, tb, :, 0:DH],
                            in_=ps.rearrange("p (h d) -> p h d", h=4),
                        )

                # ---- phase B: global sliding-window stream ----
                # step n: S+exp for step n, PV for step n-OFF.
                # blocks (qb, pair) in order; 16 kt steps per block.
                OFF = off  # ktp (2-kt) units of PV arrears
                blocks = [(qb, pair) for qb in range(QB) for pair in range(PAIRS)]
                nsteps = len(blocks) * KTP if "b" in parts else 0
                pvs_of = {}
                equeue = []
                oqueue = []

                def emit_s_exp(bi, ktp):
                    qb, pair = blocks[bi]
                    qsl = slice(qb * 512, (qb + 1) * 512)
                    # Both heads' S matmuls are emitted hh-interleaved: the
                    # two heads occupy disjoint 64-row halves of the PE array
                    # (tile_position auto-derives (0,0) / (64,0) from the
                    # operand base partitions), so back-to-back mms on
                    # alternating row-tiles can stream concurrently.
                    s_ps = [
                        psum.tile(
                            [128, 1024], F32, name=f"s{hh}_{bi}_{ktp}", tag="s"
                        )
                        for hh in range(2)
                    ]
                    for j in range(2):
                        kt = 2 * ktp + j
                        ksl = slice(kt * 128, (kt + 1) * 128)
                        for hh in range(2):
                            base = hh * DH
                            nc.tensor.matmul(
                                s_ps[hh][:, j * 512 : (j + 1) * 512],
                                kTt[pair][base : base + DH, ksl],
                                qT[pair][base : base + DH, qsl],
                                start=True,
                                stop=True,
                            )
                    ets = []
                    for hh in range(2):
                        e_t = epool.tile(
                            [128, 1024], CDT, name=f"e{hh}_{bi}_{ktp}", tag="e"
                        )
                        if exp_op == "exp" and hh == 1 and ktp in dve_ktps:
                            # DVE Schraudolph exp: offloads the saturated
                            # ACT engine; also frees hh1's s-slot in
                            # parallel with hh0's ACT exp.
                            nc.vector._custom_dve(
                                EXP2_BITS_ANT,
                                out=e_t[:].bitcast(mybir.dt.int16),
                                in0=s_ps[hh][:],
                                s0=SCALE * 128.0 / _LN2,
                                s1=_MAGIC + (16256.0 - _SCHC),
                                imm2=_MAGIC,
                            )
                        else:
                            nc.scalar.activation(
                                out=e_t[:],
                                in_=s_ps[hh][:],
                                func=(
                                    mybir.ActivationFunctionType.Exp
                                    if exp_op == "exp"
                                    else mybir.ActivationFunctionType.Copy
                                ),
                                bias=0.0,
                                scale=SCALE if exp_op == "exp" else 1.0,
                            )
                        ets.append(e_t)
                    return ets

                def finish_block(bi):
                    qb, pair = blocks[bi]
                    qsl = slice(qb * 512, (qb + 1) * 512)
                    pv = pvs_of.pop(bi)
                    for hh in range(2):
                        # PE-free normalization: the denominator arrives
                        # already replicated on PSUM rows 64..127 (ones-
                        # columns in V_aug), so this is just DVE reciprocal
                        # + one fused multiply.  The baseline's ones-matmul
                        # broadcast put the PE behind this DVE/ACT chain at
                        # every block boundary (the FIFO engine queue stalls
                        # on it), and those gaps re-throttled the PE clock
                        # via the HAM MID window.
                        # reciprocal_approx_fast is a bitwise-seed op; feed
                        # it from SBUF (a plain copy converts the PSUM
                        # accumulator format to clean IEEE fp32 -- reading
                        # PSUM directly NaN'd rare elements on HW).
                        den_s = spool.tile(
                            [DH, 512], F32, name=f"dn{hh}_{bi}", tag="den_s"
                        )
                        nc.vector.tensor_copy(
                            out=den_s[:], in_=pv[hh][DH : 2 * DH, :]
                        )
                        recip_f = spool.tile(
                            [DH, 512], F32, name=f"rf{hh}_{bi}", tag="recip_f"
                        )
                        nc.vector.reciprocal_approx_fast(
                            out=recip_f[:], in_=den_s[:]
                        )
                        nc.vector.tensor_mul(
                            out=aT[pair][hh * DH : (hh + 1) * DH, qsl],
                            in0=pv[hh][0:DH, :],
                            in1=recip_f[:],
                        )
                    if pair == 1 and "c" in parts:
                        # out-projection for this q-block (both pairs
                        # done): queue the 8 chunks and drain them 1-2
                        # per step of the NEXT block.  Emitting all 16
                        # matmuls in one burst spikes PE power (observed
                        # 0.81GHz DVFS clamp in those windows) and
                        # clusters the aT dependency.
                        for tbl in range(4):
                            for eb in range(2):
                                oqueue.append((qb * 4 + tbl, eb))

                def emit_oproj(tb, eb):
                    ps = psum.tile(
                        [128, 512],
                        F32,
                        name=f"ps_o{tb}{eb}",
                        tag="pv",
                        bufs=4,
                    )
                    for kt2 in range(2):
                        nc.tensor.matmul(
                            ps[:],
                            aT[kt2][:, tb * 128 : (tb + 1) * 128],
                            wo_t[:, kt2, eb * 512 : (eb + 1) * 512],
                            start=(kt2 == 0),
                            stop=(kt2 == 1),
                        )
                    o_s = opool.tile(
                        [128, 512], ODT, name=f"o_{tb}{eb}", tag="o"
                    )
                    # DVE copy: the ACT engine is the phase-B wall (128 exps
                    # ~1.1us each); with the finish chain shrunk to 3 ops the
                    # DVE has the headroom for these.
                    nc.vector.tensor_copy(out=o_s[:], in_=ps[:])
                    nc.sync.dma_start(
                        out=out_d.ap()[
                            tb * 128 : (tb + 1) * 128,
                            eb * 512 : (eb + 1) * 512,
                        ],
                        in_=o_s[:],
                    )

                for n in range(nsteps + OFF):
                    if n < nsteps:
                        bi, ktp = divmod(n, KTP)
                        if ktp == 0:
                            pvs_of[bi] = [
                                psum.tile(
                                    [128, 512],
                                    F32,
                                    name=f"pv{hh}_{bi}",
                                    tag="pv",
                                    bufs=4,
                                )
                                for hh in range(2)
                            ]
                        equeue.append(emit_s_exp(bi, ktp))
                    if n >= OFF:
                        bi, ktp = divmod(n - OFF, KTP)
                        qb, pair = blocks[bi]
                        cur = equeue.pop(0)
                        pv = pvs_of[bi]
                        for hh in range(2):
                            for j in range(2):
                                kt = 2 * ktp + j
                                nc.tensor.matmul(
                                    pv[hh][:, :],
                                    v_t[:, kt, 2 * pair + hh, :],
                                    cur[hh][:, j * 512 : (j + 1) * 512],
                                    start=(kt == 0),
                                    stop=(kt == KT - 1),
                                )
                        if ktp == KTP - 1:
                            finish_block(bi)
                    if n >= OFF and oqueue:
                        emit_oproj(*oqueue.pop(0))
                while oqueue:
                    emit_oproj(*oqueue.pop(0))

            if loop_n:
                with tc.For_i(0, loop_n, 1):
                    _body()
            else:
                for rep in range(nrep):
                    _body()

    nc.compile()
    return nc


_NC = None


def get_nc():
    global _NC
    if _NC is None:
        _NC = build_nc()
    return _NC


def make_in_maps(query, key, value, Wq, bq, Wk, bk, Wv, bv, Wo, bo):
    query = np.asarray(query, dtype=np.float32)
    key = np.asarray(key, dtype=np.float32)
    value = np.asarray(value, dtype=np.float32)
    Wq = np.asarray(Wq, dtype=np.float32)
    Wk = np.asarray(Wk, dtype=np.float32)
    Wv = np.asarray(Wv, dtype=np.float32)
    Wo = np.asarray(Wo, dtype=np.float32)
    bq = np.asarray(bq, dtype=np.float32)
    bk = np.asarray(bk, dtype=np.float32)

    if USE_BF16_X:
        import ml_dtypes

        xd = ml_dtypes.bfloat16
    else:
        xd = np.float32
    def shuf_x(x):
        # [T,E] -> xT [E,T] -> [p=128, quart, et, c] partition-major so
        # each per-quart DMA is one contiguous 8KB run per partition.
        xt = x.T.reshape(ET, 128, 4, 512)
        return np.ascontiguousarray(
            xt.transpose(1, 2, 0, 3).reshape(128, 4 * ET * 512).astype(xd)
        )

    def shuf_w(w):
        # [E, DLOC] -> [p=128, et, m] partition-major.
        return np.ascontiguousarray(
            w.reshape(ET, 128, DLOC)
            .transpose(1, 0, 2)
            .reshape(128, ET * DLOC)
            .astype(xd)
        )

    xq = [shuf_x(query[b]) for b in range(2)]
    xk = [shuf_x(key[b]) for b in range(2)]
    xv = [shuf_x(value[b]) for b in range(2)]
    wqt = [shuf_w(Wq[g * DLOC : (g + 1) * DLOC, :].T) for g in range(4)]
    wkt = [shuf_w(Wk[g * DLOC : (g + 1) * DLOC, :].T) for g in range(4)]
    wvt = [shuf_w(Wv[g * DLOC : (g + 1) * DLOC, :].T) for g in range(4)]
    wot = [
        np.ascontiguousarray(
            Wo[:, g * DLOC : (g + 1) * DLOC]
            .T.reshape(2, 128, E)
            .transpose(1, 0, 2)
            .reshape(128, 2 * E)
            .astype(xd)
        )
        for g in range(4)
    ]

    in_maps = []
    for c in range(8):
        b, g = divmod(c, 4)
        in_maps.append(
            {
                "xq": xq[b],
                "xk": xk[b],
                "xv": xv[b],
                "wqt": wqt[g],
                "wkt": wkt[g],
                "wvt": wvt[g],
                "wot": wot[g],
                "bq": np.ascontiguousarray(
                    bq[g * DLOC : (g + 1) * DLOC].reshape(2, 128).T
                ),
                "bk": np.ascontiguousarray(
                    bk[g * DLOC : (g + 1) * DLOC].reshape(2, 128).T
                ),
            }
        )
    return in_maps


class _CachedSpmd:
    """Build the PJRT executable for an 8-core SPMD bass module once and
    reuse it across calls (run_bass_kernel_spmd re-jits every call)."""

    def __init__(self, nc, n_cores=8):
        import jax
        import numpy as _np
        from jax.sharding import Mesh, PartitionSpec
        from jax.experimental.shard_map import shard_map
        import concourse.mybir as _mybir
        from concourse import bass2jax

        bass2jax.install_neuronx_cc_hook()
        self.n_cores = n_cores
        partition_name = (
            nc.partition_id_tensor.name if nc.partition_id_tensor else None
        )
        in_names, out_names, out_avals, zero_shapes = [], [], [], []
        for alloc in nc.m.functions[0].allocations:
            if not isinstance(alloc, _mybir.MemoryLocationSet):
                continue
            name = alloc.memorylocations[0].name
            if alloc.kind == "ExternalInput":
                if name == partition_name:
                    continue
                in_names.append(name)
            elif alloc.kind == "ExternalOutput":
                out_names.append(name)
                shape = tuple(alloc.tensor_shape)
                dtype = _mybir.dt.np(alloc.dtype)
                out_avals.append(jax.core.ShapedArray(shape, dtype))
                zero_shapes.append((shape, dtype))
        self.in_names = list(in_names)
        self.out_names = list(out_names)
        self.out_avals = out_avals
        self.zero_shapes = zero_shapes
        n_params = len(in_names)
        n_outs = len(out_names)
        all_names = list(in_names) + list(out_names)
        if partition_name is not None:
            all_names.append(partition_name)

        def _body(*args):
            operands = list(args)
            if partition_name is not None:
                operands.append(bass2jax.partition_id_tensor())
            outs = bass2jax._bass_exec_p.bind(
                *operands,
                out_avals=tuple(out_avals),
                in_names=tuple(all_names),
                out_names=tuple(out_names),
                lowering_input_output_aliases=(),
                sim_require_finite=True,
                sim_require_nnan=True,
                nc=nc,
            )
            return tuple(outs)

        devices = jax.devices()[:n_cores]
        mesh = Mesh(_np.asarray(devices), ("core",))
        in_specs = (PartitionSpec("core"),) * (n_params + n_outs)
        out_specs = (PartitionSpec("core"),) * n_outs
        self.sharded = jax.jit(
            shard_map(
                _body,
                mesh=mesh,
                in_specs=in_specs,
                out_specs=out_specs,
                check_rep=False,
            ),
            donate_argnums=tuple(range(n_params, n_params + n_outs)),
            keep_unused=True,
        )

    def concat_inputs(self, in_maps):
        import numpy as _np

        return [
            _np.concatenate([_np.asarray(m[name]) for m in in_maps], axis=0)
            for name in self.in_names
        ]

    def zeros(self):
        import numpy as _np

        return [
            _np.zeros((self.n_cores * s[0], *s[1:]), d)
            for s, d in self.zero_shapes
        ]

    def run(self, in_maps):
        import numpy as _np

        out_arrs = self.sharded(*self.concat_inputs(in_maps), *self.zeros())
        return [
            {
                name: _np.asarray(out_arrs[i]).reshape(
                    self.n_cores, *self.out_avals[i].shape
                )[c]
                for i, name in enumerate(self.out_names)
            }
            for c in range(self.n_cores)
        ]


_EXEC = None


def get_exec():
    global _EXEC
    if _EXEC is None:
        _EXEC = _CachedSpmd(get_nc())
    return _EXEC


def kernel(query, key, value, Wq, bq, Wk, bk, Wv, bv, Wo, bo):
    in_maps = make_in_maps(query, key, value, Wq, bq, Wk, bk, Wv, bv, Wo, bo)
    results = get_exec().run(in_maps)
    parts = [results[c]["out"] for c in range(8)]
    Wo_np = np.asarray(Wo, dtype=np.float32)
    bv_np = np.asarray(bv, dtype=np.float32)
    bo_np = np.asarray(bo, dtype=np.float32)
    corr = bo_np + Wo_np @ bv_np
    out = np.empty((2, T, E), dtype=np.float32)
    for b in range(2):
        acc = parts[b * 4].astype(np.float32)
        for g in range(1, 4):
            acc = acc + parts[b * 4 + g]
        out[b] = acc + corr[None, :]
    return out



# revision 33
# speedup vs baseline: 1.0957x; 1.0957x over previous
"""Cross-attention (B=2, T=2048, E=1024, H=16, Dh=64) on 8 trn2 NeuronCores.

Sharding: core c = b*4 + g  ->  batch b in {0,1}, head-group g in {0..3}
(4 heads per core).  Each core computes its heads' Q/K/V projections,
attention, and a partial out-projection; the host sums the 4 head-group
partials per batch (the tensor-parallel all-reduce, done at unshard
time) and adds the bias correction  bo + Wo @ bv.

Layout strategy: activations are fed to the device pre-transposed
([E, T] instead of [T, E]) so every matmul operand has its contraction
dim on partitions with no on-chip transposes.  Weights are fed as
Wq/Wk/Wv row-slices transposed ([E, 256]) and the Wo column-slice
transposed ([256, E]).

Attention math per head (no max-subtraction needed: |scores| <~ 10):
  S^T[k,q]  = K @ Q^T                (lhsT = K^T tile, rhs = Q^T tile;
              two heads share the PE array via row tiling at
              partition bases 0 and 64)
  Ex        = exp(S^T / 8)           (ScalarE, scale folded into exp,
              1024-wide over two k-tiles to amortize op overhead)
  PV[d,q]   = sum_k V_aug[k,d] * Ex[k,q]   (V_aug has a ones column ->
              row 64 accumulates the softmax denominator)
  A^T[d,q]  = PV[0:64,q] * (1 / PV[64,q])  (denominator broadcast to 64
              partitions with a K=1 matmul against a ones row)
  out      += A^T.T @ WoT_slice      (partial; host sums over groups)

Optimization history (NTFF-profiled NEFF exec time, core 0):
  f32r baseline                          ~426 us  (rel err 3.8e-04)
  + on-chip ones-column fill (the DMA    ~407 us
    broadcast was 8192 4B packets that
    blocked input streaming for ~76us)
  + all-bf16 matmul tiles (f32r tripped  ~314 us  (rel err 6.1e-03)
    the PE power throttle: util clamped
    to ~50-65% for 60% of the kernel)
  + approx reciprocal + spread out-proj  ~301 us  (rel err 6.3e-03)
    over following steps (power spikes)
    + bf16 output DMA
  + partition-major pre-shuffled inputs  ~292 us
    (1 descriptor/partition DMAs)
  + ACT-engine out-proj copies, OFF=5    ~288 us
Remaining wall: PE active ~250us of the ~290us span, with the DVFS
power throttle holding matmuls at ~1.35GHz (379ns/512-row matmul vs
213ns at the 2.4GHz peak; all 8 cores share the package power budget).
CA_PREC=f32r restores the full-precision path (~1.4x slower).
"""

import numpy as np

import concourse.bacc as bacc
import concourse.mybir as mybir
import concourse.tile as tile
from concourse.bass_utils import run_bass_kernel_spmd

# ---- custom DVE op: Schraudolph exp to bf16 bit pattern, one DVE pass ----
# j = round(x*(128/ln2)*SCALE + (16256 - C)) computed in fp32 via the
# magic-addend trick (adding 1.5*2^23 rounds to integer ULP); stored to an
# int16 view of the bf16 e-tile, whose bit pattern IS bf16(exp(x*SCALE)).
# Element rel err ~4.6% RMS (mantissa linear-interp sawtooth); diluted by
# sqrt(f) when only a fraction f of k-tiles use it, and further by softmax
# averaging.  Offloads exp from the saturated ACT engine to the DVE.
import concourse.dve_ops as _dve_ops
from concourse.dve_spec import C0 as _C0, C1 as _C1, C2 as _C2, Spec as _Spec, Src0 as _Src0
from concourse.dve_ops import DveOp as _DveOp

_LN2 = float(np.log(2.0))
_MAGIC = float(1.5 * 2**23)
_SCHC = 7.0  # Schraudolph bias: centers the sawtooth so DVE-exp tiles are
# unbiased relative to exact ACT-exp tiles (the bias, not the noise, is
# what hurts when the two are mixed within one softmax row)


def _exp2_bits_ref(in0, in1, s0, s1, imm2):
    t = (in0.astype(np.float32) * np.float32(s0)).astype(np.float32)
    t = (t + np.float32(s1)).astype(np.float32)
    return (t - np.float32(imm2)).astype(np.float32)


def _exp8_bits_ref(in0, in1, s0, s1, imm2):
    t = (in0.astype(np.float32) * np.float32(s0)).astype(np.float32)
    t = (t + np.float32(s1)).astype(np.float32)
    t = (t - np.float32(imm2)).astype(np.float32)
    return np.maximum(t, 0).astype(np.float32)


def _register_dve_op(name, spec, shas):
    for _op in _dve_ops.OPS:
        if _op.name == name:
            return _op
    _op = _DveOp(name, spec, subdim=False, uops_sha=shas)
    _dve_ops.OPS.append(_op)
    _dve_ops.CUSTOM_DVE_SPECS[_op.name] = _op.spec
    _dve_ops._SUB_OPCODE_FOR_NAME[_op.name] = (
        _dve_ops._CUSTOM_DVE_ROW_BASE + len(_dve_ops.OPS) - 1
    )
    return _op


EXP2_BITS_ANT = _register_dve_op(
    "EXP2_BITS_ANT",
    _Spec(body=(_Src0 * _C0 + _C1) - _C2, reference=_exp2_bits_ref),
    {"v3": "d08b4dae0546ff96", "v4": "78111510d711fb37"},
)
# fp8e4 variant: writes TRN float8e4 bit patterns (exp bias 7, 3-bit
# mantissa, j units 8/octave); relu clamps underflow to +0.0.
from concourse.dve_spec import relu as _relu

EXP8_BITS_ANT = _register_dve_op(
    "EXP8_BITS_ANT",
    _Spec(body=_relu((_Src0 * _C0 + _C1) - _C2), reference=_exp8_bits_ref),
    {"v3": "a251ca82b38b006d", "v4": "e55cb8db58e19942"},
)
_SCHC8 = 0.46  # fp8 sawtooth mean-centering (7.34/16)

E = 1024          # embed dim
T = 2048          # seq len (q and k)
DH = 64           # head dim
DLOC = 256        # per-core projected dim (4 heads * 64)
ET = E // 128     # 8 contraction tiles over embed
KT = T // 128     # 16 k-tiles
KTP = KT // 2     # 8 k-tile pairs (exp processes 1024-wide)
TB = T // 128     # 16 t-blocks
QB = T // 512     # 4 q-blocks of 512
PAIRS = 2         # head pairs per core (2 heads share the 128 partitions)
SCALE = float(1.0 / np.sqrt(DH))

F32 = mybir.dt.float32
# Matmul compute dtype.  f32r streams 1 row/cycle like bf16, but the
# fp32 MAC power draw trips the DVFS throttle (measured: util clamped
# to ~50-65% for 60% of the kernel, ~2x on every matmul).  bf16 runs
# unthrottled and halves input DMA; measured rel err stays ~4e-3
# (gate 2e-2).  CA_PREC=f32r restores the full-precision path.
import os as _os

PREC = _os.environ.get("CA_PREC", "bf16")
MM_DT = mybir.dt.bfloat16 if PREC == "bf16" else mybir.dt.float32r


CDT = MM_DT  # dtype for every tile that feeds a matmul
USE_BF16_X = PREC == "bf16"
XDT = MM_DT
# fp8e4 attention-value path: Ex and V_aug in float8e4 enable a DoubleRow
# PV matmul (2 k-tiles per instruction, ~2x PV throughput).  exp gets a -2
# bias folded in so Ex stays within TRN e4m3 range (max 240); the softmax
# ratio cancels the common factor exactly.
# NOTE: fp8 measured at ~3.9e-2 output error -- attention output is a
# weighted average whose magnitude shrinks ~1/sqrt(Neff), exactly as fast
# as quantization noise averages down, so fp8's ~3.5% element error hits
# the output nearly 1:1.  Stays off; the gate is 2e-2.
PV8 = PREC == "bf16" and _os.environ.get("CA_PV", "bf16") == "fp8"
EDT = mybir.dt.float8e4 if PV8 else CDT
EXPB = -2.0 if PV8 else 0.0
# Output dtype: bf16 halves the 8MB/core output DMA and the end-of-
# kernel drain; the host upcasts before summing the 4 partials.
ODT = mybir.dt.bfloat16 if PREC == "bf16" else F32


def build_nc(nrep=1, loop_n=0, timing=False, parts="xavbc", exp_op="exp", epool_bufs=10, off=5):
    # timing=True: big inputs become Internal (device-resident, unbound) so
    # the measurement harness ships ~nothing per dispatch; loop_n>0 wraps
    # the body in an on-device For_i loop to amplify exec time over the
    # dispatch noise.
    nc = bacc.Bacc("TRN2", target_bir_lowering=False, debug=False)

    # All big inputs are host-pre-shuffled to partition-major layouts so
    # every DMA lands as one large contiguous descriptor per partition
    # (8x fewer packets; the 2KB+ descriptors hit peak HBM throughput).
    big = "Internal" if timing else "ExternalInput"
    xq_d = nc.dram_tensor("xq", [128, 4 * ET * 512], XDT, kind=big)
    xk_d = nc.dram_tensor("xk", [128, 4 * ET * 512], XDT, kind=big)
    xv_d = nc.dram_tensor("xv", [128, 4 * ET * 512], XDT, kind=big)
    wq_d = nc.dram_tensor("wqt", [128, ET * DLOC], XDT, kind=big)
    wk_d = nc.dram_tensor("wkt", [128, ET * DLOC], XDT, kind=big)
    wv_d = nc.dram_tensor("wvt", [128, ET * DLOC], XDT, kind=big)
    wo_d = nc.dram_tensor("wot", [128, 2 * E], CDT, kind=big)
    bq_d = nc.dram_tensor("bq", [128, 2], F32, kind="ExternalInput")
    bk_d = nc.dram_tensor("bk", [128, 2], F32, kind="ExternalInput")
    out_d = nc.dram_tensor("out", [T, E], ODT, kind="ExternalOutput")

    import concourse.bass as bass
    from contextlib import ExitStack

    if EXPB != 0.0 and (F32, EXPB) not in nc.const_aps.aps:
        # register a const AP for the exp bias (activation converts float
        # biases to [128,1] const APs; only 0.0/1.0 are pre-registered)
        _bias_t = nc.alloc_sbuf_tensor(f"const-float32-{EXPB}", [128, 1], F32)
        nc.gpsimd.memset(_bias_t.ap(), EXPB)
        nc.const_aps.aps[(F32, EXPB)] = _bias_t.ap()
        nc.all_engine_barrier()

    with tile.TileContext(nc) as tc:
        with ExitStack() as ctx:
            persist = ctx.enter_context(tc.tile_pool(name="persist", bufs=1))
            wpool = ctx.enter_context(tc.tile_pool(name="wpool", bufs=1))
            xpool = ctx.enter_context(tc.tile_pool(name="xpool", bufs=6))
            epool = ctx.enter_context(tc.tile_pool(name="epool", bufs=epool_bufs))
            spool = ctx.enter_context(tc.tile_pool(name="spool", bufs=2))
            opool = ctx.enter_context(tc.tile_pool(name="opool", bufs=2))
            # PSUM: 8 banks.  tag "s" = 2 slots of [128,1024] (2 banks
            # each: S-pair outputs; also projections and bc, briefly); tag
            # "pv" = 4 single-bank slots (PV accumulators for two in-flight
            # blocks, and out-proj accumulators).
            psum = ctx.enter_context(
                tc.tile_pool(name="psum", bufs=2, space="PSUM")
            )

            # Of the 8 ktp steps per block, how many of hh1's exps go to the
            # DVE Schraudolph op (ACT does the rest).  4 -> 25% of all exps.
            n8 = int(_os.environ.get("CA_DVE_EXP", "8"))
            if PREC != "bf16":
                n8 = 0
            dve_ktps = (
                set() if n8 <= 0 else {int(round(i * 8 / n8)) % 8 for i in range(n8)}
            )
            WARM = int(_os.environ.get("CA_WARM", "0"))

            def _body():
                # ---- PE warm-up ----
                # The HAM clock gate starts at K=4/8 (1.2GHz) and needs
                # ~3.4us of sustained PE activity to open up; the real work
                # is DMA-gated for the first ~10us.  Dummy matmuls on
                # zeroed tiles warm the clock so phase A runs at 2.4GHz.
                if WARM:
                    wu_w = persist.tile([128, 128], F32, name="wu_w", tag="wu_w")
                    wu_x = persist.tile([128, 512], F32, name="wu_x", tag="wu_x")
                    nc.vector.memset(wu_w[:], 0.0)
                    nc.vector.memset(wu_x[:], 0.0)
                    for i in range(WARM):
                        wu_ps = psum.tile(
                            [128, 512], F32, name=f"wu{i}", tag="pv", bufs=4
                        )
                        nc.tensor.matmul(
                            wu_ps[:], wu_w[:], wu_x[:], start=True, stop=True
                        )

                # ---- persistent tiles ----
                qT = [
                    persist.tile([128, T], CDT, name=f"qT{p}", tag=f"qT{p}")
                    for p in range(PAIRS)
                ]
                kTt = [
                    persist.tile([128, T], CDT, name=f"kT{p}", tag=f"kT{p}")
                    for p in range(PAIRS)
                ]
                aT = [
                    persist.tile([128, T], CDT, name=f"aT{p}", tag=f"aT{p}")
                    for p in range(PAIRS)
                ]
                # V is augmented with 64 ones-columns (cols DH..2*DH-1): the
                # PV matmul then emits the softmax denominator REPLICATED on
                # PSUM partitions 64..127 for free (matmul cost is N cycles
                # regardless of M), so normalization needs no partition
                # broadcast at all -- DVE reciprocal on [64,512] is barely
                # slower than [1,512] (it was single-lane before).
                v_t = persist.tile(
                    [128, KT, 4, 2 * DH], EDT, name="v_t", tag="v_t"
                )
                bq_t = persist.tile([128, 2], F32, name="bq_t", tag="bq_t")
                bk_t = persist.tile([128, 2], F32, name="bk_t", tag="bk_t")

                # Fill the ones-columns on-chip (a DMA broadcast here is
                # scattered 4B packets that monopolize the DMA rings; memset
                # can't emit f32r/fp8, so stage F32 and DVE-copy over).
                ones_col = persist.tile(
                    [128, KT * 4 * DH], F32, name="ones_col", tag="ones_col"
                )
                nc.vector.memset(ones_col[:], 1.0)
                nc.vector.tensor_copy(
                    out=v_t[:, :, :, DH : 2 * DH],
                    in_=ones_col.rearrange("p (k h d) -> p k h d", k=KT, h=4),
                )

                wq_t = wpool.tile([128, ET, DLOC], XDT, name="wq_t", tag="wq")
                wk_t = wpool.tile([128, ET, DLOC], XDT, name="wk_t", tag="wk")
                wv_t = wpool.tile([128, ET, DLOC], XDT, name="wv_t", tag="wv")
                wo_t = wpool.tile([128, 2, E], CDT, name="wo_t", tag="wo")

                # ---- phase A: projections (quarter x-tiles of 512 cols) ----
                # q/k/v interleaved per quart so all three input streams
                # prefetch ahead (the baseline's serial q,k,v order left a
                # ~3.9us PE gap waiting for xv -- one HAM MID window is
                # enough to re-throttle the PE clock to 1.2GHz).
                nc.sync.dma_start(
                    out=wq_t[:],
                    in_=wq_d.ap().rearrange("p (n m) -> p n m", n=ET),
                )
                nc.sync.dma_start(out=bq_t[:], in_=bq_d.ap())
                nc.sync.dma_start(
                    out=wk_t[:],
                    in_=wk_d.ap().rearrange("p (n m) -> p n m", n=ET),
                )
                nc.sync.dma_start(out=bk_t[:], in_=bk_d.ap())
                nc.sync.dma_start(
                    out=wv_t[:],
                    in_=wv_d.ap().rearrange("p (n m) -> p n m", n=ET),
                )
                nc.sync.dma_start(
                    out=wo_t[:],
                    in_=wo_d.ap().rearrange("p (n m) -> p n m", n=2),
                )
                QW = ET * 512
                for quart in range(4 if "x" in parts else 0):
                    csl = slice(quart * 512, (quart + 1) * 512)
                    xhs = {}
                    for nm, x_d in (("q", xq_d), ("k", xk_d), ("v", xv_d)):
                        xh = xpool.tile(
                            [128, ET, 512], XDT, name=f"x_{nm}{quart}", tag="x"
                        )
                        nc.sync.dma_start(
                            out=xh[:],
                            in_=x_d.ap()[
                                :, quart * QW : (quart + 1) * QW
                            ].rearrange("p (n c) -> p n c", n=ET),
                        )
                        xhs[nm] = xh
                    if "a" not in parts:
                        continue
                    for nm, w_t, b_t, dst in (
                        ("q", wq_t, bq_t, qT),
                        ("k", wk_t, bk_t, kTt),
                    ):
                        xh = xhs[nm]
                        for m in range(PAIRS):
                            ps = psum.tile(
                                [128, 512],
                                F32,
                                name=f"ps_{nm}{quart}{m}",
                                tag="s",
                            )
                            for et in range(ET):
                                nc.tensor.matmul(
                                    ps[:],
                                    w_t[:, et, m * 128 : (m + 1) * 128],
                                    xh[:, et, :],
                                    start=(et == 0),
                                    stop=(et == ET - 1),
                                )
                            nc.vector.tensor_scalar_add(
                                out=dst[m][:, csl],
                                in0=ps[:],
                                scalar1=b_t[:, m : m + 1],
                            )
                    xh = xhs["v"]
                    for tbl in range(4 if "v" in parts else 0):
                        tb = quart * 4 + tbl
                        ps = psum.tile([128, DLOC], F32, name=f"ps_v{tb}", tag="s")
                        for et in range(ET):
                            nc.tensor.matmul(
                                ps[:],
                                xh[:, et, tbl * 128 : (tbl + 1) * 128],
                                wv_t[:, et, :],
                                start=(et == 0),
                                stop=(et == ET - 1),
                            )
                        nc.vector.tensor_copy(
                            out=v_t[:, tb, :, 0:DH],
                            in_=ps.rearrange("p (h d) -> p h d", h=4),
                        )

                # ---- phase B: global sliding-window stream ----
                # step n: S+exp for step n, PV for step n-OFF.
                # blocks (qb, pair) in order; 16 kt steps per block.
                OFF = off  # ktp (2-kt) units of PV arrears
                blocks = [(qb, pair) for qb in range(QB) for pair in range(PAIRS)]
                nsteps = len(blocks) * KTP if "b" in parts else 0
                pvs_of = {}
                equeue = []
                oqueue = []

                def emit_s_exp(bi, ktp):
                    qb, pair = blocks[bi]
                    qsl = slice(qb * 512, (qb + 1) * 512)
                    # Both heads' S matmuls are emitted hh-interleaved: the
                    # two heads occupy disjoint 64-row halves of the PE array
                    # (tile_position auto-derives (0,0) / (64,0) from the
                    # operand base partitions), so back-to-back mms on
                    # alternating row-tiles can stream concurrently.
                    s_ps = [
                        psum.tile(
                            [128, 1024], F32, name=f"s{hh}_{bi}_{ktp}", tag="s"
                        )
                        for hh in range(2)
                    ]
                    for j in range(2):
                        kt = 2 * ktp + j
                        ksl = slice(kt * 128, (kt + 1) * 128)
                        for hh in range(2):
                            base = hh * DH
                            nc.tensor.matmul(
                                s_ps[hh][:, j * 512 : (j + 1) * 512],
                                kTt[pair][base : base + DH, ksl],
                                qT[pair][base : base + DH, qsl],
                                start=True,
                                stop=True,
                            )
                    ets = []
                    for hh in range(2):
                        e_t = epool.tile(
                            [128, 1024], EDT, name=f"e{hh}_{bi}_{ktp}", tag="e"
                        )
                        if exp_op == "exp" and hh == 1 and ktp in dve_ktps:
                            # DVE Schraudolph exp: offloads the saturated
                            # ACT engine; also frees hh1's s-slot in
                            # parallel with hh0's ACT exp, which lets the
                            # scheduler co-issue the two heads' S matmuls.
                            if PV8:
                                nc.vector._custom_dve(
                                    EXP8_BITS_ANT,
                                    out=e_t[:].bitcast(mybir.dt.int8),
                                    in0=s_ps[hh][:],
                                    s0=SCALE * 8.0 / _LN2,
                                    s1=_MAGIC
                                    + (56.0 - _SCHC8)
                                    + EXPB * 8.0 / _LN2,
                                    imm2=_MAGIC,
                                )
                            else:
                                nc.vector._custom_dve(
                                    EXP2_BITS_ANT,
                                    out=e_t[:].bitcast(mybir.dt.int16),
                                    in0=s_ps[hh][:],
                                    s0=SCALE * 128.0 / _LN2,
                                    s1=_MAGIC + (16256.0 - _SCHC),
                                    imm2=_MAGIC,
                                )
                        else:
                            nc.scalar.activation(
                                out=e_t[:],
                                in_=s_ps[hh][:],
                                func=(
                                    mybir.ActivationFunctionType.Exp
                                    if exp_op == "exp"
                                    else mybir.ActivationFunctionType.Copy
                                ),
                                bias=EXPB if exp_op == "exp" else 0.0,
                                scale=SCALE if exp_op == "exp" else 1.0,
                            )
                        ets.append(e_t)
                    return ets

                def finish_block(bi):
                    qb, pair = blocks[bi]
                    qsl = slice(qb * 512, (qb + 1) * 512)
                    pv = pvs_of.pop(bi)
                    for hh in range(2):
                        # PE-free normalization: the denominator arrives
                        # already replicated on PSUM rows 64..127 (ones-
                        # columns in V_aug), so this is just DVE reciprocal
                        # + one fused multiply.  The baseline's ones-matmul
                        # broadcast put the PE behind this DVE/ACT chain at
                        # every block boundary (the FIFO engine queue stalls
                        # on it), and those gaps re-throttled the PE clock
                        # via the HAM MID window.
                        # reciprocal_approx_fast is a bitwise-seed op; feed
                        # it from SBUF (a plain copy converts the PSUM
                        # accumulator format to clean IEEE fp32 -- reading
                        # PSUM directly NaN'd rare elements on HW).
                        den_s = spool.tile(
                            [DH, 512], F32, name=f"dn{hh}_{bi}", tag="den_s"
                        )
                        nc.scalar.activation(
                            out=den_s[:],
                            in_=pv[hh][DH : 2 * DH, :],
                            func=mybir.ActivationFunctionType.Copy,
                            bias=0.0,
                            scale=1.0,
                        )
                        recip_f = spool.tile(
                            [DH, 512], F32, name=f"rf{hh}_{bi}", tag="recip_f"
                        )
                        nc.vector.reciprocal_approx_fast(
                            out=recip_f[:], in_=den_s[:]
                        )
                        nc.vector.tensor_mul(
                            out=aT[pair][hh * DH : (hh + 1) * DH, qsl],
                            in0=pv[hh][0:DH, :],
                            in1=recip_f[:],
                        )
                    if pair == 1 and "c" in parts:
                        # out-projection for this q-block (both pairs
                        # done): queue the 8 chunks and drain them 1-2
                        # per step of the NEXT block.  Emitting all 16
                        # matmuls in one burst spikes PE power (observed
                        # 0.81GHz DVFS clamp in those windows) and
                        # clusters the aT dependency.
                        for tbl in range(4):
                            for eb in range(2):
                                oqueue.append((qb * 4 + tbl, eb))

                def emit_oproj(tb, eb):
                    ps = psum.tile(
                        [128, 512],
                        F32,
                        name=f"ps_o{tb}{eb}",
                        tag="pv",
                        bufs=4,
                    )
                    for kt2 in range(2):
                        nc.tensor.matmul(
                            ps[:],
                            aT[kt2][:, tb * 128 : (tb + 1) * 128],
                            wo_t[:, kt2, eb * 512 : (eb + 1) * 512],
                            start=(kt2 == 0),
                            stop=(kt2 == 1),
                        )
                    o_s = opool.tile(
                        [128, 512], ODT, name=f"o_{tb}{eb}", tag="o"
                    )
                    # ACT copy: with half the exps offloaded to the DVE
                    # Schraudolph op, the ACT engine has the headroom and
                    # the DVE (which also runs the finish chains) does not.
                    nc.scalar.activation(
                        out=o_s[:],
                        in_=ps[:],
                        func=mybir.ActivationFunctionType.Copy,
                        bias=0.0,
                        scale=1.0,
                    )
                    nc.sync.dma_start(
                        out=out_d.ap()[
                            tb * 128 : (tb + 1) * 128,
                            eb * 512 : (eb + 1) * 512,
                        ],
                        in_=o_s[:],
                    )

                for n in range(nsteps + OFF):
                    if n < nsteps:
                        bi, ktp = divmod(n, KTP)
                        if ktp == 0:
                            pvs_of[bi] = [
                                psum.tile(
                                    [128, 512],
                                    F32,
                                    name=f"pv{hh}_{bi}",
                                    tag="pv",
                                    bufs=4,
                                )
                                for hh in range(2)
                            ]
                        equeue.append(emit_s_exp(bi, ktp))
                    if n >= OFF:
                        bi, ktp = divmod(n - OFF, KTP)
                        qb, pair = blocks[bi]
                        cur = equeue.pop(0)
                        pv = pvs_of[bi]
                        for hh in range(2):
                            if PV8:
                                # fp8 DoubleRow: one matmul contracts both
                                # k-tiles of this step (2 fp8 weights/cell,
                                # 2 MACs/cycle) -- ~2x PV throughput.  The
                                # e-tile's [p, (j q)] layout is exactly the
                                # [Ki, Ko=2, N] interleave DoubleRow wants.
                                nc.tensor.matmul(
                                    pv[hh][:, :],
                                    v_t[:, 2 * ktp : 2 * ktp + 2, 2 * pair + hh, :],
                                    cur[hh].rearrange("p (o q) -> p o q", o=2),
                                    start=(ktp == 0),
                                    stop=(ktp == KTP - 1),
                                    perf_mode=mybir.MatmulPerfMode.DoubleRow,
                                )
                            else:
                                for j in range(2):
                                    kt = 2 * ktp + j
                                    nc.tensor.matmul(
                                        pv[hh][:, :],
                                        v_t[:, kt, 2 * pair + hh, :],
                                        cur[hh][:, j * 512 : (j + 1) * 512],
                                        start=(kt == 0),
                                        stop=(kt == KT - 1),
                                    )
                        if ktp == KTP - 1:
                            finish_block(bi)
                    if n >= OFF and oqueue:
                        emit_oproj(*oqueue.pop(0))
                while oqueue:
                    emit_oproj(*oqueue.pop(0))

            if loop_n:
                with tc.For_i(0, loop_n, 1):
                    _body()
            else:
                for rep in range(nrep):
                    _body()

    nc.compile()
    return nc


_NC = None


def get_nc():
    global _NC
    if _NC is None:
        _NC = build_nc()
    return _NC


def make_in_maps(query, key, value, Wq, bq, Wk, bk, Wv, bv, Wo, bo):
    query = np.asarray(query, dtype=np.float32)
    key = np.asarray(key, dtype=np.float32)
    value = np.asarray(value, dtype=np.float32)
    Wq = np.asarray(Wq, dtype=np.float32)
    Wk = np.asarray(Wk, dtype=np.float32)
    Wv = np.asarray(Wv, dtype=np.float32)
    Wo = np.asarray(Wo, dtype=np.float32)
    bq = np.asarray(bq, dtype=np.float32)
    bk = np.asarray(bk, dtype=np.float32)

    if USE_BF16_X:
        import ml_dtypes

        xd = ml_dtypes.bfloat16
    else:
        xd = np.float32
    def shuf_x(x):
        # [T,E] -> xT [E,T] -> [p=128, quart, et, c] partition-major so
        # each per-quart DMA is one contiguous 8KB run per partition.
        xt = x.T.reshape(ET, 128, 4, 512)
        return np.ascontiguousarray(
            xt.transpose(1, 2, 0, 3).reshape(128, 4 * ET * 512).astype(xd)
        )

    def shuf_w(w):
        # [E, DLOC] -> [p=128, et, m] partition-major.
        return np.ascontiguousarray(
            w.reshape(ET, 128, DLOC)
            .transpose(1, 0, 2)
            .reshape(128, ET * DLOC)
            .astype(xd)
        )

    xq = [shuf_x(query[b]) for b in range(2)]
    xk = [shuf_x(key[b]) for b in range(2)]
    xv = [shuf_x(value[b]) for b in range(2)]
    wqt = [shuf_w(Wq[g * DLOC : (g + 1) * DLOC, :].T) for g in range(4)]
    wkt = [shuf_w(Wk[g * DLOC : (g + 1) * DLOC, :].T) for g in range(4)]
    wvt = [shuf_w(Wv[g * DLOC : (g + 1) * DLOC, :].T) for g in range(4)]
    wot = [
        np.ascontiguousarray(
            Wo[:, g * DLOC : (g + 1) * DLOC]
            .T.reshape(2, 128, E)
            .transpose(1, 0, 2)
            .reshape(128, 2 * E)
            .astype(xd)
        )
        for g in range(4)
    ]

    in_maps = []
    for c in range(8):
        b, g = divmod(c, 4)
        in_maps.append(
            {
                "xq": xq[b],
                "xk": xk[b],
                "xv": xv[b],
                "wqt": wqt[g],
                "wkt": wkt[g],
                "wvt": wvt[g],
                "wot": wot[g],
                "bq": np.ascontiguousarray(
                    bq[g * DLOC : (g + 1) * DLOC].reshape(2, 128).T
                ),
                "bk": np.ascontiguousarray(
                    bk[g * DLOC : (g + 1) * DLOC].reshape(2, 128).T
                ),
            }
        )
    return in_maps


class _CachedSpmd:
    """Build the PJRT executable for an 8-core SPMD bass module once and
    reuse it across calls (run_bass_kernel_spmd re-jits every call)."""

    def __init__(self, nc, n_cores=8):
        import jax
        import numpy as _np
        from jax.sharding import Mesh, PartitionSpec
        from jax.experimental.shard_map import shard_map
        import concourse.mybir as _mybir
        from concourse import bass2jax

        bass2jax.install_neuronx_cc_hook()
        self.n_cores = n_cores
        partition_name = (
            nc.partition_id_tensor.name if nc.partition_id_tensor else None
        )
        in_names, out_names, out_avals, zero_shapes = [], [], [], []
        for alloc in nc.m.functions[0].allocations:
            if not isinstance(alloc, _mybir.MemoryLocationSet):
                continue
            name = alloc.memorylocations[0].name
            if alloc.kind == "ExternalInput":
                if name == partition_name:
                    continue
                in_names.append(name)
            elif alloc.kind == "ExternalOutput":
                out_names.append(name)
                shape = tuple(alloc.tensor_shape)
                dtype = _mybir.dt.np(alloc.dtype)
                out_avals.append(jax.core.ShapedArray(shape, dtype))
                zero_shapes.append((shape, dtype))
        self.in_names = list(in_names)
        self.out_names = list(out_names)
        self.out_avals = out_avals
        self.zero_shapes = zero_shapes
        n_params = len(in_names)
        n_outs = len(out_names)
        all_names = list(in_names) + list(out_names)
        if partition_name is not None:
            all_names.append(partition_name)

        def _body(*args):
            operands = list(args)
            if partition_name is not None:
                operands.append(bass2jax.partition_id_tensor())
            outs = bass2jax._bass_exec_p.bind(
                *operands,
                out_avals=tuple(out_avals),
                in_names=tuple(all_names),
                out_names=tuple(out_names),
                lowering_input_output_aliases=(),
                sim_require_finite=True,
                sim_require_nnan=True,
                nc=nc,
            )
            return tuple(outs)

        devices = jax.devices()[:n_cores]
        mesh = Mesh(_np.asarray(devices), ("core",))
        in_specs = (PartitionSpec("core"),) * (n_params + n_outs)
        out_specs = (PartitionSpec("core"),) * n_outs
        self.sharded = jax.jit(
            shard_map(
                _body,
                mesh=mesh,
                in_specs=in_specs,
                out_specs=out_specs,
                check_rep=False,
            ),
            donate_argnums=tuple(range(n_params, n_params + n_outs)),
            keep_unused=True,
        )

    def concat_inputs(self, in_maps):
        import numpy as _np

        return [
            _np.concatenate([_np.asarray(m[name]) for m in in_maps], axis=0)
            for name in self.in_names
        ]

    def zeros(self):
        import numpy as _np

        return [
            _np.zeros((self.n_cores * s[0], *s[1:]), d)
            for s, d in self.zero_shapes
        ]

    def run(self, in_maps):
        import numpy as _np

        out_arrs = self.sharded(*self.concat_inputs(in_maps), *self.zeros())
        return [
            {
                name: _np.asarray(out_arrs[i]).reshape(
                    self.n_cores, *self.out_avals[i].shape
                )[c]
                for i, name in enumerate(self.out_names)
            }
            for c in range(self.n_cores)
        ]


_EXEC = None


def get_exec():
    global _EXEC
    if _EXEC is None:
        _EXEC = _CachedSpmd(get_nc())
    return _EXEC


def kernel(query, key, value, Wq, bq, Wk, bk, Wv, bv, Wo, bo):
    in_maps = make_in_maps(query, key, value, Wq, bq, Wk, bk, Wv, bv, Wo, bo)
    results = get_exec().run(in_maps)
    parts = [results[c]["out"] for c in range(8)]
    Wo_np = np.asarray(Wo, dtype=np.float32)
    bv_np = np.asarray(bv, dtype=np.float32)
    bo_np = np.asarray(bo, dtype=np.float32)
    corr = bo_np + Wo_np @ bv_np
    out = np.empty((2, T, E), dtype=np.float32)
    for b in range(2):
        acc = parts[b * 4].astype(np.float32)
        for g in range(1, 4):
            acc = acc + parts[b * 4 + g]
        out[b] = acc + corr[None, :]
    return out

